# revision 3
# baseline (speedup 1.0000x reference)
"""Trainium2 Bass kernel for nn_RelFeatFusion (2-layer encoder over [B=512,K=32,D=1936],
2-layer decoder over the transposed [n=32,B=512] grouping, fusion head).

Strategy: two SPMD launches on 8 cores.
  Phase 1 (encoder): data-parallel over images (64 images = 2048 tokens/core).
  Host reshuffle:    [B,K] -> [K,B] regrouping of the encoder output.
  Phase 2 (decoder+fusion): data-parallel over labels (4 labels = 2048 tokens/core).

On-chip layout: activations are feature-major ("transposed", [feat, tok]) so every
matmul contracts along the partition dim. D padded 1936->2048, each head padded
242->256 so all tiles are clean 128s. The residual stream X lives in bf16; all
large matmuls run in fp8e4m3 with DoubleRow perf mode (contraction pairs of
128-row tiles -> 2x PE throughput). Weights are pre-transposed/padded, scaled by
16 (fp8 subnormal avoidance) and cast to fp8 on the host; the 1/16 is folded into
downstream scales (exp scale for QK, softmax denominators for V via sel16=16,
ScalarE activation scale elsewhere). exp() carries a -2 bias (cancels in softmax)
so unnormalized probabilities stay below fp8e4m3's +-240 range. LayerNorm
statistics and per-token broadcasts are small PE matmuls (ones-column reductions
and f32r rank-1 broadcast outer products)."""
import math
import numpy as np
import ml_dtypes

import concourse.bass as bass
import concourse.mybir as mybir
import concourse.tile as tile
from concourse.bass import ts, ds
from concourse.bass_utils import run_bass_kernel_spmd

F32 = mybir.dt.float32
F32R = mybir.dt.float32r
BF16 = mybir.dt.bfloat16
F8 = mybir.dt.float8e4
BF = ml_dtypes.bfloat16
F8NP = ml_dtypes.float8_e4m3
AF = mybir.ActivationFunctionType
OP = mybir.AluOpType
DR = mybir.MatmulPerfMode.DoubleRow

B, K, D, NH, DFF = 512, 32, 1936, 8, 2048
LENC, LDEC = 2, 2
HD = D // NH          # 242
Dp = 2048
HDp = 256
EPS = 1e-5
NCORES = 8
T = 2048              # tokens per core
CH = 512              # chunk tokens
SCALE = 1.0 / math.sqrt(HD)
WS = 16.0             # host-side fp8 weight scale
IWS = 1.0 / WS
SC8 = SCALE / (WS * WS)   # exp scale absorbing Q and K weight scales
EXPB = -2.0               # exp bias margin (cancels in softmax)

# ----------------------------------------------------------------- wait splitting

def _split_excess_waits(nc, limit=1):
    """walrus rejects >1 semaphore wait on most instruction formats; move the
    excess onto NoOps inserted just before the instruction (same engine)."""
    for fn in nc.m.functions:
        for blk in fn.blocks:
            new = []
            dirty = False
            for ins in list(blk.instructions):
                si = getattr(ins, "sync_info", None)
                waits = list(si.on_wait) if si is not None else []
                if len(waits) > limit:
                    dirty = True
                    k = 0
                    while len(waits) - k > limit:
                        nop = mybir.InstNoOp(name=f"{ins.name}_ws{k}", ins=[], outs=[])
                        nop.engine = ins.engine
                        nop.sync_info = mybir.SyncInfo(on_wait=waits[k:k + 1], on_update=[])
                        new.append(nop)
                        k += 1
                    si.on_wait = waits[k:]
                new.append(ins)
            if dirty:
                blk.instructions = new


# ----------------------------------------------------------------- host weight prep

def _hp_map():
    """out-feature index map for head padding: padded row h*256+j <- h*242+j."""
    m = np.full(Dp, -1, dtype=np.int64)
    for h in range(NH):
        m[h * HDp: h * HDp + HD] = np.arange(h * HD, (h + 1) * HD)
    return m

HPM = _hp_map()

def _wt_pad(w, in_map="id", out_map="id"):
    """w: [out_real, in_real] f32 -> padded WT [Dp_in, Dp_out] f32.
    WT[i_pad, o_pad] = w[o, i].  in_map/out_map: 'id' | 'hp'."""
    out_real, in_real = w.shape
    WT = np.zeros((Dp, Dp), dtype=np.float32)

    if out_map == "id":
        ocols = np.arange(out_real)
        osrc = np.arange(out_real)
    elif out_map == "hp":
        ocols = np.nonzero(HPM >= 0)[0]
        osrc = HPM[ocols]
    else:
        raise ValueError(out_map)

    if in_map == "id":
        irows = np.arange(in_real)
        isrc = np.arange(in_real)
    elif in_map == "hp":
        irows = np.nonzero(HPM >= 0)[0]
        isrc = HPM[irows]
    else:
        raise ValueError(in_map)

    WT[np.ix_(irows, ocols)] = w[np.ix_(osrc, isrc)].T
    return WT

def _f8(WT):
    return np.clip(WT * WS, -240.0, 240.0).astype(F8NP)

def _lhsT_stream(WT):
    """[Dp_in, Dp_out] -> [16, 128, 16, 128] f8: arr[m,cp,ci,col]=WT[ci*128+cp, m*128+col]."""
    return np.ascontiguousarray(
        _f8(WT).reshape(16, 128, 16, 128).transpose(2, 1, 0, 3))

def _rhs_stream(WT):
    """[Dp_in, Dp_out] -> [4, 128, 16, 512] f8: arr[n,cp,ci,col]=WT[ci*128+cp, n*512+col]."""
    return np.ascontiguousarray(
        _f8(WT).reshape(16, 128, 4, 512).transpose(2, 1, 0, 3))

def _timing_signal():
    pos = np.arange(B, dtype=np.float32)
    num_ts = D // 2
    log_incr = np.float32(np.log(1e4).astype(np.float32) / max(num_ts - 1, 1))
    inv = np.exp(np.arange(num_ts, dtype=np.float32) * -log_incr)
    scaled = pos[:, None] * inv[None, :]
    sig = np.concatenate([np.sin(scaled), np.cos(scaled)], -1)  # [B, D]
    out = np.zeros((Dp, B), dtype=np.float32)
    out[:D] = sig.T
    return out.astype(BF)                                      # [Dp, 512]

def _enc_mask():
    base = np.zeros((128, 128), dtype=np.float32)
    for i in range(4):
        base[i * 32:(i + 1) * 32, i * 32:(i + 1) * 32] = 1.0
    return np.tile(base, (1, NH)).reshape(128, NH, 128).astype(BF)

def _prep_weights(inp):
    """Build all padded/streamed weight arrays (shared across cores)."""
    w = {}
    for pfx, L in (("enc", LENC), ("dec", LDEC)):
        qkv_w = np.asarray(inp[pfx + "_qkv_w"], np.float32)
        qkv_b = np.asarray(inp[pfx + "_qkv_b"], np.float32)
        out_w = np.asarray(inp[pfx + "_out_w"], np.float32)
        out_b = np.asarray(inp[pfx + "_out_b"], np.float32)
        ff1_w = np.asarray(inp[pfx + "_ff1_w"], np.float32)
        ff1_b = np.asarray(inp[pfx + "_ff1_b"], np.float32)
        ff2_w = np.asarray(inp[pfx + "_ff2_w"], np.float32)
        ff2_b = np.asarray(inp[pfx + "_ff2_b"], np.float32)
        assert not np.any(qkv_b) and not np.any(out_b) and not np.any(ff1_b) \
            and not np.any(ff2_b), "nonzero biases unsupported by this kernel build"
        for l in range(L):
            w[f"{pfx}{l}_wq"] = _lhsT_stream(_wt_pad(qkv_w[l, 0:D], "id", "hp"))
            w[f"{pfx}{l}_wk"] = _lhsT_stream(_wt_pad(qkv_w[l, D:2 * D], "id", "hp"))
            w[f"{pfx}{l}_wv"] = _rhs_stream(_wt_pad(qkv_w[l, 2 * D:], "id", "hp"))
            w[f"{pfx}{l}_wo"] = _lhsT_stream(_wt_pad(out_w[l], "hp", "id"))
            w[f"{pfx}{l}_w1"] = _lhsT_stream(_wt_pad(ff1_w[l], "id", "id"))
            w[f"{pfx}{l}_w2"] = _lhsT_stream(_wt_pad(ff2_w[l], "id", "id"))
    for nm in ("enc_ln1", "enc_ln2", "dec_ln"):
        assert np.all(np.asarray(inp[nm + "_g"]) == 1.0), "ln gamma != 1 unsupported"
        assert not np.any(np.asarray(inp[nm + "_b"])), "ln beta != 0 unsupported"

    fuse_w = np.asarray(inp["fuse_w"], np.float32)
    fuse_b = np.asarray(inp["fuse_b"], np.float32)
    att1_w = np.asarray(inp["att1_w"], np.float32)
    att1_b = np.asarray(inp["att1_b"], np.float32)
    att2_w = np.asarray(inp["att2_w"], np.float32)
    att2_b = np.asarray(inp["att2_b"], np.float32)
    assert not np.any(fuse_b) and not np.any(att1_b) and not np.any(att2_b), \
        "nonzero fusion biases unsupported"
    w["wfa"] = _lhsT_stream(_wt_pad(fuse_w[:, :D], "id", "id"))
    w["wfb"] = _lhsT_stream(_wt_pad(fuse_w[:, D:], "id", "id"))
    w["wa1"] = _lhsT_stream(_wt_pad(att1_w, "id", "id"))
    w["wa2"] = _lhsT_stream(_wt_pad(att2_w, "id", "id"))
    w["mask"] = _enc_mask()
    w["pos"] = _timing_signal()
    return w


# ----------------------------------------------------------------- device builders

def _re(ap):
    return ap.rearrange("(ci cp) t -> cp ci t", cp=128)

def _ln_device(nc, p, X):
    """In-place layernorm over the feature (partition) dim of X [128,16,512] bf16.
    Specialized to ln gamma==1, beta==0 (asserted host-side): X = (X-mean)*rstd.
    Pad rows (1936..2047) end up holding -mean*rstd, which is harmless: every
    downstream weight stream has zero rows there and stats exclude them."""
    Sq = p["sqp"].tile([128, 16, 512], BF16, tag="sq")
    nc.vector.tensor_mul(Sq[:], X[:], X[:])
    ps_s = p["ppr"].tile([1, 512], F32, tag="st")
    ps_q = p["ppr"].tile([1, 512], F32, tag="st")
    sel = p["sel"]
    for c in range(16):
        sl = sel[:, 0:1] if c < 15 else sel[:, 1:2]
        nc.tensor.matmul(ps_s[:], sl, X[:, c, :], start=(c == 0), stop=(c == 15))
        nc.tensor.matmul(ps_q[:], sl, Sq[:, c, :], start=(c == 0), stop=(c == 15))
    rows = p["rows"]
    mean = rows.tile([1, 512], F32, tag="r1")
    nc.vector.tensor_scalar_mul(mean[:], ps_s[:], 1.0 / D)
    var = rows.tile([1, 512], F32, tag="r2")
    nc.vector.tensor_scalar_mul(var[:], ps_q[:], 1.0 / D)
    msq = rows.tile([1, 512], F32, tag="r3")
    nc.vector.tensor_mul(msq[:], mean[:], mean[:])
    nc.vector.tensor_sub(var[:], var[:], msq[:])
    nc.scalar.activation(var[:], var[:], AF.Sqrt, bias=p["epsr"][0:1, 0:1])
    rstd = rows.tile([1, 512], F32, tag="r4")
    nc.vector.reciprocal(rstd[:], var[:])
    rstd_r = rows.tile([1, 512], F32R, tag="r5")
    nc.vector.tensor_copy(rstd_r[:], rstd[:])
    shn = rows.tile([1, 512], F32, tag="r6")
    nc.vector.tensor_mul(shn[:], mean[:], rstd[:])
    nc.vector.tensor_scalar_mul(shn[:], shn[:], -1.0)
    shn_r = rows.tile([1, 512], F32R, tag="r7")
    nc.vector.tensor_copy(shn_r[:], shn[:])
    o1 = p["ones128r"]
    p1 = p["pps"].tile([128, 512], F32, tag="bc")
    nc.tensor.matmul(p1[:], o1[0:1, :], rstd_r[:], start=True, stop=True)
    p2 = p["pps"].tile([128, 512], F32, tag="bc")
    nc.tensor.matmul(p2[:], o1[0:1, :], shn_r[:], start=True, stop=True)
    for c in range(16):
        nc.vector.tensor_tensor(X[:, c, :], X[:, c, :], p1[:], OP.mult)
        nc.vector.tensor_tensor(X[:, c, :], X[:, c, :], p2[:], OP.add)


def _proj_dr(nc, p, w_d, src8, consume, wtag="w"):
    """psum[m] = sum_j DRmm(w_d[m][:, 2j:2j+2, :], src8[:, 2j:2j+2, :]); consume(m, psum).
    Weights are fp8 scaled by WS; consume must fold in IWS."""
    for m in range(16):
        wt = p["wp"].tile([128, 16, 128], F8, tag=wtag)
        nc.sync.dma_start(wt[:], w_d[m])
        ps = p["pp"].tile([128, 512], F32, tag="p")
        for j in range(8):
            nc.tensor.matmul(ps[:], wt[:, ds(2 * j, 2), :], src8[:, ds(2 * j, 2), :],
                             start=(j == 0), stop=(j == 7), perf_mode=DR)
        consume(m, ps)


def _attn_enc(nc, p, QT, KT, V, OT, maskb):
    """QT/KT [128,16,512] f8 (x16 scale), V [128,4,Dp] f8 (x16), OT f8 out (natural)."""
    for g in range(4):
        Pg = p["pgp"].tile([128, NH, 128], F8, tag="Pg")
        for h in range(NH):
            S = p["pps"].tile([128, 512], F32, tag="S")
            nc.tensor.matmul(S[:, 0:128], KT[:, ds(2 * h, 2), ts(g, 128)],
                             QT[:, ds(2 * h, 2), ts(g, 128)],
                             start=True, stop=True, perf_mode=DR)
            nc.scalar.activation(Pg[:, h, :], S[:, 0:128], AF.Exp, scale=SC8, bias=EXPB)
        nc.vector.tensor_tensor(Pg[:], Pg[:], maskb[:], OP.mult)
        sel16 = p["sel16"]
        bcs = []
        for half in (0, 1):
            dn = p["ppr"].tile([1, 512], F32, tag="st")
            nc.tensor.matmul(dn[:], sel16[:, 0, :], Pg[:, 4 * half:4 * half + 4, :],
                             start=True, stop=True)
            rc = p["rows"].tile([1, 512], F32, tag=f"r{half}")
            nc.vector.reciprocal(rc[:], dn[:])
            rc_r = p["rows"].tile([1, 512], F32R, tag=f"rr{half}")
            nc.vector.tensor_copy(rc_r[:], rc[:])
            bcp = p["pps"].tile([128, 512], F32, tag="bc")
            nc.tensor.matmul(bcp[:], p["ones128r"][0:1, :], rc_r[:],
                             start=True, stop=True)
            bcb = p["bcs"].tile([128, 512], F32, tag="bcs")
            nc.vector.tensor_copy(bcb[:], bcp[:])
            bcs.append(bcb)
        for h in range(NH):
            for mm in (0, 1):
                po = p["pps"].tile([128, 512], F32, tag="S")
                nc.tensor.matmul(po[:, 0:128], V[:, g, ds((2 * h + mm) * 128, 128)],
                                 Pg[:, h, :], start=True, stop=True)
                nc.vector.tensor_tensor(
                    OT[:, 2 * h + mm, ts(g, 128)], po[:, 0:128],
                    bcs[h // 4][:, ds((h % 4) * 128, 128)], OP.mult)


def _attn_dec(nc, p, QT, KT, V, OT):
    sel16 = p["sel16"]
    for h in range(NH):
        P = p["pgp"].tile([128, 4, 512], F8, tag="Pd")
        for kt in range(4):
            S = p["pps"].tile([128, 512], F32, tag="S")
            nc.tensor.matmul(S[:], KT[:, ds(2 * h, 2), ts(kt, 128)],
                             QT[:, ds(2 * h, 2), :], start=True, stop=True,
                             perf_mode=DR)
            nc.scalar.activation(P[:, kt, :], S[:], AF.Exp, scale=SC8, bias=EXPB)
        dn = p["ppr"].tile([1, 512], F32, tag="st")
        for j in (0, 1):
            nc.tensor.matmul(dn[:], sel16[:], P[:, ds(2 * j, 2), :],
                             start=(j == 0), stop=(j == 1), perf_mode=DR)
        rc = p["rows"].tile([1, 512], F32, tag="r1")
        nc.vector.reciprocal(rc[:], dn[:])
        rc_r = p["rows"].tile([1, 512], F32R, tag="r2")
        nc.vector.tensor_copy(rc_r[:], rc[:])
        bcp = p["pps"].tile([128, 512], F32, tag="bc")
        nc.tensor.matmul(bcp[:], p["ones128r"][0:1, :], rc_r[:], start=True, stop=True)
        bcb = p["bcs"].tile([128, 512], F32, tag="bcs")
        nc.vector.tensor_copy(bcb[:], bcp[:])
        for mm in (0, 1):
            po = p["pps"].tile([128, 512], F32, tag="S")
            for j in (0, 1):
                nc.tensor.matmul(po[:], V[:, ds(2 * j, 2), ds((2 * h + mm) * 128, 128)],
                                 P[:, ds(2 * j, 2), :], start=(j == 0), stop=(j == 1),
                                 perf_mode=DR)
            nc.vector.tensor_tensor(OT[:, 2 * h + mm, :], po[:], bcb[:], OP.mult)


def build_phase(phase, n_layers=2, n_chunks=4, fusion=True, reps=1):
    """phase: 'enc' or 'dec'. reps>1 wraps the whole body in a hardware loop
    (identical re-execution, for wall-clock timing of device time)."""
    enc = phase == "enc"
    nc = bass.Bass()
    x_d = nc.dram_tensor("x", [Dp, T], BF16, kind="ExternalInput")
    wd = {}
    for l in range(n_layers):
        for nm in ("wq", "wk", "wo", "w1", "w2"):
            shp = [16, 128, 16, 128]
            wd[f"{l}_{nm}"] = nc.dram_tensor(f"{phase}{l}_{nm}", shp, F8,
                                             kind="ExternalInput")
        wd[f"{l}_wv"] = nc.dram_tensor(f"{phase}{l}_wv", [4, 128, 16, 512], F8,
                                       kind="ExternalInput")
    if enc:
        mask_d = nc.dram_tensor("mask", [128, NH, 128], BF16, kind="ExternalInput")
        y_d = nc.dram_tensor("y", [Dp, T], BF16, kind="ExternalOutput")
    else:
        pos_d = nc.dram_tensor("pos", [Dp, B], BF16, kind="ExternalInput")
        if fusion:
            for nm in ("wfa", "wfb", "wa1", "wa2"):
                wd[nm] = nc.dram_tensor(nm, [16, 128, 16, 128], F8,
                                        kind="ExternalInput")
            y_d = nc.dram_tensor("o", [2 * D, T], BF16, kind="ExternalOutput")
        else:
            y_d = nc.dram_tensor("y", [Dp, T], BF16, kind="ExternalOutput")

    from contextlib import ExitStack
    with tile.TileContext(nc) as tc, ExitStack() as ctx:
        p = {}
        const = ctx.enter_context(tc.tile_pool(name="const", bufs=1))
        p["xp"] = ctx.enter_context(tc.tile_pool(name="xp", bufs=2))
        p["castp"] = ctx.enter_context(tc.tile_pool(name="castp", bufs=1))
        p["cast2p"] = ctx.enter_context(tc.tile_pool(name="cast2p", bufs=1))
        p["sqp"] = ctx.enter_context(tc.tile_pool(name="sqp", bufs=1))
        p["hp"] = ctx.enter_context(tc.tile_pool(name="hp", bufs=1))
        p["qtp"] = ctx.enter_context(tc.tile_pool(name="qtp", bufs=1))
        p["ktp"] = ctx.enter_context(tc.tile_pool(name="ktp", bufs=1))
        p["vp"] = ctx.enter_context(tc.tile_pool(name="vp", bufs=1))
        p["otp"] = ctx.enter_context(tc.tile_pool(name="otp", bufs=1))
        p["wp"] = ctx.enter_context(tc.tile_pool(name="wp", bufs=4))
        p["wvp"] = ctx.enter_context(tc.tile_pool(name="wvp", bufs=2))
        p["pgp"] = ctx.enter_context(tc.tile_pool(name="pgp", bufs=2))
        p["rows"] = ctx.enter_context(tc.tile_pool(name="rows", bufs=1))
        p["osp"] = ctx.enter_context(tc.tile_pool(name="osp", bufs=2))
        p["o2p"] = ctx.enter_context(tc.tile_pool(name="o2p", bufs=2))
        p["bcs"] = ctx.enter_context(tc.tile_pool(name="bcs", bufs=2))
        p["pp"] = ctx.enter_context(tc.tile_pool(name="pp", bufs=2, space="PSUM"))
        p["ppr"] = ctx.enter_context(tc.tile_pool(name="ppr", bufs=2, space="PSUM"))
        p["pps"] = ctx.enter_context(tc.tile_pool(name="pps", bufs=2, space="PSUM"))

        # constants
        sel = const.tile([128, 2], BF16)
        nc.vector.memset(sel[:, 0:1], 1.0)
        nc.vector.memset(sel[:, 1:2], 0.0)
        nc.vector.memset(sel[0:16, 1:2], 1.0)
        p["sel"] = sel
        sel16 = const.tile([128, 2, 1], F8)
        nc.vector.memset(sel16[:], WS)
        p["sel16"] = sel16
        onesf = const.tile([1, 512], F32)
        nc.vector.memset(onesf[:], 1.0)
        o128r = const.tile([1, 128], F32R)
        nc.vector.tensor_copy(o128r[:], onesf[:, 0:128])
        p["ones128r"] = o128r
        epsr = const.tile([1, 1], F32)
        nc.vector.memset(epsr[:], EPS)
        p["epsr"] = epsr
        maskb = None
        if enc:
            maskb = const.tile([128, NH, 128], BF16)
            nc.sync.dma_start(maskb[:], mask_d[:])
        else:
            posc = const.tile([128, 16, 512], BF16)
            nc.sync.dma_start(posc[:], _re(pos_d[:]))

        from contextlib import nullcontext
        loop_cm = tc.For_i(0, reps, 1) if reps > 1 else nullcontext()
        with loop_cm:
          for chk in range(n_chunks):
            X = p["xp"].tile([128, 16, 512], BF16, tag="X")
            nc.sync.dma_start(X[:], _re(x_d[:, ts(chk, 512)]))

            for l in range(n_layers):
                # ---- qkv inputs (fp8, x1 scale)
                if enc:
                    x8 = p["castp"].tile([128, 16, 512], F8, tag="x8")
                    nc.vector.tensor_copy(x8[:], X[:])
                    x8q = x8
                else:
                    x8q = p["castp"].tile([128, 16, 512], F8, tag="x8")
                    nc.vector.tensor_tensor(x8q[:], X[:], posc[:], OP.add)
                    x8 = p["cast2p"].tile([128, 16, 512], F8, tag="x8v")
                    nc.vector.tensor_copy(x8[:], X[:])

                QT = p["qtp"].tile([128, 16, 512], F8, tag="QT")
                KT = p["ktp"].tile([128, 16, 512], F8, tag="KT")
                _proj_dr(nc, p, wd[f"{l}_wq"], x8q,
                         lambda m, ps, _Q=QT: nc.vector.tensor_copy(_Q[:, m, :], ps[:]))
                _proj_dr(nc, p, wd[f"{l}_wk"], x8q,
                         lambda m, ps, _K=KT: nc.vector.tensor_copy(_K[:, m, :], ps[:]))

                V = p["vp"].tile([128, 4, Dp], F8, tag="V")
                for n in range(4):
                    wt = p["wvp"].tile([128, 16, 512], F8, tag="wv")
                    nc.sync.dma_start(wt[:], wd[f"{l}_wv"][n])
                    for mt in range(4):
                        ps = p["pp"].tile([128, 512], F32, tag="p")
                        for j in range(8):
                            nc.tensor.matmul(ps[:], x8[:, ds(2 * j, 2), ts(mt, 128)],
                                             wt[:, ds(2 * j, 2), :],
                                             start=(j == 0), stop=(j == 7),
                                             perf_mode=DR)
                        nc.vector.tensor_copy(V[:, mt, ts(n, 512)], ps[:])

                OT = p["otp"].tile([128, 16, 512], F8, tag="OT")
                if enc:
                    _attn_enc(nc, p, QT, KT, V, OT, maskb)
                else:
                    _attn_dec(nc, p, QT, KT, V, OT)

                # ---- out-proj + residual (scale IWS on ScalarE, add on DVE)
                def _res_consume(m, ps, _X=X):
                    os_ = p["osp"].tile([128, 512], BF16, tag="os")
                    nc.scalar.activation(os_[:], ps[:], AF.Copy, scale=IWS)
                    nc.vector.tensor_tensor(_X[:, m, :], _X[:, m, :], os_[:], OP.add)
                _proj_dr(nc, p, wd[f"{l}_wo"], OT, _res_consume)
                # ---- LN1 (enc) / LN (dec)
                _ln_device(nc, p, X)
                # ---- FFN
                x8t = p["castp"].tile([128, 16, 512], F8, tag="x8")
                nc.vector.tensor_copy(x8t[:], X[:])
                H = p["hp"].tile([128, 16, 512], F8, tag="H")
                _proj_dr(nc, p, wd[f"{l}_w1"], x8t,
                         lambda m, ps, _H=H: nc.scalar.activation(
                             _H[:, m, :], ps[:], AF.Relu, scale=IWS))
                _proj_dr(nc, p, wd[f"{l}_w2"], H, _res_consume)
                if enc:
                    _ln_device(nc, p, X)

            if enc or not fusion:
                nc.sync.dma_start(_re(y_d[:, ts(chk, 512)]), X[:])
            else:
                # ---------------- fusion head (chunk == one label, 512 occurrences)
                yb8 = p["castp"].tile([128, 16, 512], F8, tag="x8")
                nc.vector.tensor_copy(yb8[:], X[:])
                d0b8 = p["cast2p"].tile([128, 16, 512], F8, tag="x8v")
                nc.vector.memset(d0b8[:, :, 0:1], 0.0)
                nc.vector.tensor_copy(d0b8[:, :, 1:512], yb8[:, :, 0:511])

                diffb = p["qtp"].tile([128, 16, 512], F8, tag="QT")
                for m in range(16):
                    wta = p["wp"].tile([128, 16, 128], F8, tag="w")
                    nc.sync.dma_start(wta[:], wd["wfa"][m])
                    wtb = p["wp"].tile([128, 16, 128], F8, tag="w")
                    nc.sync.dma_start(wtb[:], wd["wfb"][m])
                    ps = p["pp"].tile([128, 512], F32, tag="p")
                    for j in range(8):
                        nc.tensor.matmul(ps[:], wta[:, ds(2 * j, 2), :],
                                         d0b8[:, ds(2 * j, 2), :],
                                         start=(j == 0), stop=False, perf_mode=DR)
                    for j in range(8):
                        nc.tensor.matmul(ps[:], wtb[:, ds(2 * j, 2), :],
                                         yb8[:, ds(2 * j, 2), :],
                                         start=False, stop=(j == 7), perf_mode=DR)
                    nc.scalar.activation(diffb[:, m, :], ps[:], AF.Copy, scale=IWS)

                t1b = p["ktp"].tile([128, 16, 512], F8, tag="KT")
                _proj_dr(nc, p, wd["wa1"], diffb,
                         lambda m, ps, _t=t1b: nc.scalar.activation(
                             _t[:, m, :], ps[:], AF.Tanh, scale=IWS))
                d2b = p["otp"].tile([128, 16, 512], F8, tag="OT")
                _proj_dr(nc, p, wd["wa2"], t1b,
                         lambda m, ps, _t=d2b: nc.scalar.activation(
                             _t[:, m, :], ps[:], AF.Tanh, scale=IWS))
                colsl = ts(chk, 512)
                nc.sync.dma_start(
                    y_d[0:1920, colsl].rearrange("(ci cp) t -> cp ci t", cp=128),
                    X[:, 0:15, :])
                nc.sync.dma_start(y_d[1920:1936, colsl], X[0:16, 15, :])
                for ci in range(16):
                    o2s = p["o2p"].tile([128, 512], BF16, tag="o2")
                    nc.vector.tensor_tensor(o2s[:, 1:512], d2b[:, ci, 1:512],
                                            X[:, ci, 0:511], OP.mult)
                    nc.vector.tensor_tensor(o2s[:, 0:1], d2b[:, ci, 0:1],
                                            X[:, ci, 0:1], OP.mult)
                    if ci < 15:
                        nc.sync.dma_start(
                            y_d[ds(1936 + ci * 128, 128), colsl], o2s[:])
                    else:
                        nc.sync.dma_start(y_d[3856:3872, colsl], o2s[0:16, :])

    _split_excess_waits(nc)
    return nc


# ----------------------------------------------------------------- host orchestration

_CACHE = {}

def _get_phase(phase, n_layers=2, n_chunks=4, fusion=True):
    key = (phase, n_layers, n_chunks, fusion)
    if key not in _CACHE:
        _CACHE[key] = build_phase(phase, n_layers, n_chunks, fusion)
    return _CACHE[key]


def _enc_inputs(w, feats):
    """feats: [B*K, D] f32. Returns per-core in_maps for phase 1."""
    FT = np.zeros((Dp, B * K), dtype=BF)
    FT[:D] = np.ascontiguousarray(feats.T).astype(BF)
    maps = []
    for c in range(NCORES):
        m = {"x": np.ascontiguousarray(FT[:, c * T:(c + 1) * T]), "mask": w["mask"]}
        for l in range(LENC):
            for nm in ("wq", "wk", "wv", "wo", "w1", "w2"):
                m[f"enc{l}_{nm}"] = w[f"enc{l}_{nm}"]
        maps.append(m)
    return maps


def _dec_inputs(w, enc_t):
    """enc_t: [Dp, B*K] bf16 (token-major i*K+j). Returns per-core in_maps."""
    E = enc_t.reshape(Dp, B, K)
    maps = []
    for c in range(NCORES):
        Y = np.ascontiguousarray(
            E[:, :, c * 4:(c + 1) * 4].transpose(0, 2, 1)).reshape(Dp, T)
        m = {"x": Y, "pos": w["pos"]}
        for l in range(LDEC):
            for nm in ("wq", "wk", "wv", "wo", "w1", "w2"):
                m[f"dec{l}_{nm}"] = w[f"dec{l}_{nm}"]
        for nm in ("wfa", "wfb", "wa1", "wa2"):
            m[nm] = w[nm]
        maps.append(m)
    return maps


def kernel(**inputs):
    inp = {k: np.asarray(v) for k, v in inputs.items()}
    feats = inp["features"].astype(np.float32)
    w = _prep_weights(inp)

    nc1 = _get_phase("enc")
    maps1 = _enc_inputs(w, feats)
    res1 = run_bass_kernel_spmd(nc1, maps1, core_ids=list(range(NCORES)))
    enc_t = np.concatenate([res1.results[c]["y"] for c in range(NCORES)], axis=1)

    nc2 = _get_phase("dec")
    maps2 = _dec_inputs(w, enc_t)
    res2 = run_bass_kernel_spmd(nc2, maps2, core_ids=list(range(NCORES)))

    out = np.empty((B * K, 2 * D), dtype=np.float32)
    out_v = out.reshape(B, K, 2 * D)
    for c in range(NCORES):
        O = res2.results[c]["o"].astype(np.float32).reshape(2 * D, 4, B)
        out_v[:, c * 4:(c + 1) * 4, :] = O.transpose(2, 1, 0)
    return out


# revision 25
# speedup vs baseline: 1.0915x; 1.0915x over previous
"""Trainium2 Bass kernel for nn_RelFeatFusion (2-layer encoder over [B=512,K=32,D=1936],
2-layer decoder over the transposed [n=32,B=512] grouping, fusion head).

Strategy: two SPMD launches on 8 cores.
  Phase 1 (encoder): data-parallel over images (64 images = 2048 tokens/core).
  Host reshuffle:    [B,K] -> [K,B] regrouping of the encoder output.
  Phase 2 (decoder+fusion): data-parallel over labels (4 labels = 2048 tokens/core).

On-chip layout: activations are feature-major ("transposed", [feat, tok]) so every
matmul contracts along the partition dim. D padded 1936->2048, each head padded
242->256 so all tiles are clean 128s. Weights are pre-transposed/padded/bf16 on
the host into the exact DMA streaming layout. The residual stream X stays f32;
matmul inputs are bf16 casts done on ScalarE (pipelined per 128-row tile behind
the LayerNorm updates so the PE never waits on a monolithic cast). Phase I/O is
bf16. LayerNorm statistics and per-token broadcasts are small PE matmuls
(ones-column reductions and f32r rank-1 broadcast outer products); the LN
normalize pass is split across DVE and GpSimd to halve its serial latency.
Attention runs a distance-1 software pipeline (scores of group g+1 issue before
the AV matmuls of group g) so softmax reciprocal chains never stall the PE."""
import math
import numpy as np
import ml_dtypes

import concourse.bass as bass
import concourse.mybir as mybir
import concourse.tile as tile
from concourse.bass import ts, ds
from concourse.bass_utils import run_bass_kernel_spmd

F32 = mybir.dt.float32
F32R = mybir.dt.float32r
BF16 = mybir.dt.bfloat16
BF = ml_dtypes.bfloat16
AF = mybir.ActivationFunctionType
OP = mybir.AluOpType

B, K, D, NH, DFF = 512, 32, 1936, 8, 2048
LENC, LDEC = 2, 2
HD = D // NH          # 242
Dp = 2048
HDp = 256
EPS = 1e-5
NCORES = 8
T = 2048              # tokens per core
CH = 512              # chunk tokens
SCALE = 1.0 / math.sqrt(HD)

# ----------------------------------------------------------------- wait splitting

def _split_excess_waits(nc, limit=1):
    """walrus rejects >1 semaphore wait on most instruction formats; move the
    excess onto NoOps inserted just before the instruction (same engine)."""
    for fn in nc.m.functions:
        for blk in fn.blocks:
            new = []
            dirty = False
            for ins in list(blk.instructions):
                si = getattr(ins, "sync_info", None)
                waits = list(si.on_wait) if si is not None else []
                if len(waits) > limit:
                    dirty = True
                    k = 0
                    while len(waits) - k > limit:
                        nop = mybir.InstNoOp(name=f"{ins.name}_ws{k}", ins=[], outs=[])
                        nop.engine = ins.engine
                        nop.sync_info = mybir.SyncInfo(on_wait=waits[k:k + 1], on_update=[])
                        new.append(nop)
                        k += 1
                    si.on_wait = waits[k:]
                new.append(ins)
            if dirty:
                blk.instructions = new


# ----------------------------------------------------------------- host weight prep

def _hp_map():
    """out-feature index map for head padding: padded row h*256+j <- h*242+j."""
    m = np.full(Dp, -1, dtype=np.int64)
    for h in range(NH):
        m[h * HDp: h * HDp + HD] = np.arange(h * HD, (h + 1) * HD)
    return m

HPM = _hp_map()

def _wt_pad(w, in_map="id", out_map="id"):
    """w: [out_real, in_real] f32 -> padded WT [Dp_in, Dp_out] f32.
    WT[i_pad, o_pad] = w[o, i].  in_map/out_map: 'id' | 'hp'."""
    out_real, in_real = w.shape
    WT = np.zeros((Dp, Dp), dtype=np.float32)

    if out_map == "id":
        ocols = np.arange(out_real)
        osrc = np.arange(out_real)
    elif out_map == "hp":
        ocols = np.nonzero(HPM >= 0)[0]
        osrc = HPM[ocols]
    else:
        raise ValueError(out_map)

    if in_map == "id":
        irows = np.arange(in_real)
        isrc = np.arange(in_real)
    elif in_map == "hp":
        irows = np.nonzero(HPM >= 0)[0]
        isrc = HPM[irows]
    else:
        raise ValueError(in_map)

    WT[np.ix_(irows, ocols)] = w[np.ix_(osrc, isrc)].T
    return WT

def _lhsT_stream(WT):
    """[Dp_in, Dp_out] -> [16, 128, 16, 128] bf16: arr[m,cp,ci,col]=WT[ci*128+cp, m*128+col]."""
    return np.ascontiguousarray(
        WT.reshape(16, 128, 16, 128).transpose(2, 1, 0, 3)).astype(BF)

def _rhs_stream(WT):
    """[Dp_in, Dp_out] -> [4, 128, 16, 512] bf16: arr[n,cp,ci,col]=WT[ci*128+cp, n*512+col]."""
    return np.ascontiguousarray(
        WT.reshape(16, 128, 4, 512).transpose(2, 1, 0, 3)).astype(BF)

def _timing_signal():
    pos = np.arange(B, dtype=np.float32)
    num_ts = D // 2
    log_incr = np.float32(np.log(1e4).astype(np.float32) / max(num_ts - 1, 1))
    inv = np.exp(np.arange(num_ts, dtype=np.float32) * -log_incr)
    scaled = pos[:, None] * inv[None, :]
    sig = np.concatenate([np.sin(scaled), np.cos(scaled)], -1)  # [B, D]
    out = np.zeros((Dp, B), dtype=np.float32)
    out[:D] = sig.T
    return out.astype(BF)                                      # [Dp, 512]

def _enc_mask():
    base = np.zeros((128, 128), dtype=np.float32)
    for i in range(4):
        base[i * 32:(i + 1) * 32, i * 32:(i + 1) * 32] = 1.0
    return np.tile(base, (1, NH)).reshape(128, NH, 128).astype(BF)

def _prep_weights(inp):
    """Build all padded/streamed weight arrays (shared across cores)."""
    w = {}
    for pfx, L in (("enc", LENC), ("dec", LDEC)):
        qkv_w = np.asarray(inp[pfx + "_qkv_w"], np.float32)
        qkv_b = np.asarray(inp[pfx + "_qkv_b"], np.float32)
        out_w = np.asarray(inp[pfx + "_out_w"], np.float32)
        out_b = np.asarray(inp[pfx + "_out_b"], np.float32)
        ff1_w = np.asarray(inp[pfx + "_ff1_w"], np.float32)
        ff1_b = np.asarray(inp[pfx + "_ff1_b"], np.float32)
        ff2_w = np.asarray(inp[pfx + "_ff2_w"], np.float32)
        ff2_b = np.asarray(inp[pfx + "_ff2_b"], np.float32)
        assert not np.any(qkv_b) and not np.any(out_b) and not np.any(ff1_b) \
            and not np.any(ff2_b), "nonzero biases unsupported by this kernel build"
        for l in range(L):
            w[f"{pfx}{l}_wq"] = _lhsT_stream(_wt_pad(qkv_w[l, 0:D], "id", "hp"))
            w[f"{pfx}{l}_wk"] = _lhsT_stream(_wt_pad(qkv_w[l, D:2 * D], "id", "hp"))
            w[f"{pfx}{l}_wv"] = _rhs_stream(_wt_pad(qkv_w[l, 2 * D:], "id", "hp"))
            w[f"{pfx}{l}_wo"] = _lhsT_stream(_wt_pad(out_w[l], "hp", "id"))
            w[f"{pfx}{l}_w1"] = _lhsT_stream(_wt_pad(ff1_w[l], "id", "id"))
            w[f"{pfx}{l}_w2"] = _lhsT_stream(_wt_pad(ff2_w[l], "id", "id"))
    for nm in ("enc_ln1", "enc_ln2", "dec_ln"):
        assert np.all(np.asarray(inp[nm + "_g"]) == 1.0), "ln gamma != 1 unsupported"
        assert not np.any(np.asarray(inp[nm + "_b"])), "ln beta != 0 unsupported"

    fuse_w = np.asarray(inp["fuse_w"], np.float32)
    fuse_b = np.asarray(inp["fuse_b"], np.float32)
    att1_w = np.asarray(inp["att1_w"], np.float32)
    att1_b = np.asarray(inp["att1_b"], np.float32)
    att2_w = np.asarray(inp["att2_w"], np.float32)
    att2_b = np.asarray(inp["att2_b"], np.float32)
    assert not np.any(fuse_b) and not np.any(att1_b) and not np.any(att2_b), \
        "nonzero fusion biases unsupported"
    w["wfa"] = _lhsT_stream(_wt_pad(fuse_w[:, :D], "id", "id"))
    w["wfb"] = _lhsT_stream(_wt_pad(fuse_w[:, D:], "id", "id"))
    w["wa1"] = _lhsT_stream(_wt_pad(att1_w, "id", "id"))
    w["wa2"] = _lhsT_stream(_wt_pad(att2_w, "id", "id"))
    w["mask"] = _enc_mask()
    w["pos"] = _timing_signal()
    return w


# ----------------------------------------------------------------- device builders

def _re(ap):
    return ap.rearrange("(ci cp) t -> cp ci t", cp=128)

# c-tiles of the X-update handled by GpSimd. Empty: GpSimd has no PSUM access
# on TRN2, and the broadcasts live in PSUM; per-tile pipelining of the
# downstream cast keeps the update off the PE critical path anyway.
GP_TILES = frozenset()

def _ln_device(nc, p, X, finish=None):
    """In-place layernorm over the feature (partition) dim of X [128,16,512] f32.
    Specialized to ln gamma==1, beta==0 (asserted host-side): X = (X-mean)*rstd.
    Pad rows (1936..2047) end up holding -mean*rstd, which is harmless: every
    downstream weight stream has zero rows there and stats exclude them.
    finish(c, X): optional per-tile hook run right after tile c is normalized
    (used to pipeline the follow-up cast / output DMA behind the update)."""
    ps_s = p["ppr"].tile([1, 512], F32, tag="st")
    ps_q = p["ppr"].tile([1, 512], F32, tag="st")
    sel = p["sel"]
    for c in range(16):
        Rb = p["rbp"].tile([128, 512], BF16, tag="rb")
        nc.vector.tensor_copy(Rb[:], X[:, c, :])
        Sq = p["sqp"].tile([128, 512], BF16, tag="sq")
        nc.vector.tensor_mul(Sq[:], Rb[:], Rb[:])
        sl = sel[:, 0:1] if c < 15 else sel[:, 1:2]
        nc.tensor.matmul(ps_s[:], sl, Rb[:], start=(c == 0), stop=(c == 15))
        nc.tensor.matmul(ps_q[:], sl, Sq[:], start=(c == 0), stop=(c == 15))
    rows = p["rows"]
    mean = rows.tile([1, 512], F32, tag="r1")            # holds -mean
    nc.vector.tensor_scalar_mul(mean[:], ps_s[:], -1.0 / D)
    var = rows.tile([1, 512], F32, tag="r2")
    nc.vector.tensor_scalar_mul(var[:], ps_q[:], 1.0 / D)
    msq = rows.tile([1, 512], F32, tag="r3")
    nc.vector.tensor_mul(msq[:], mean[:], mean[:])
    nc.vector.tensor_sub(var[:], var[:], msq[:])
    nc.scalar.activation(var[:], var[:], AF.Sqrt, bias=p["epsr"][0:1, 0:1])
    rstd = rows.tile([1, 512], F32, tag="r3")            # reuse msq slot
    nc.vector.reciprocal(rstd[:], var[:])
    rstd_r = rows.tile([1, 512], F32R, tag="rr1")
    nc.vector.tensor_copy(rstd_r[:], rstd[:])
    shn = rows.tile([1, 512], F32, tag="r2")             # reuse var slot: -mean*rstd
    nc.vector.tensor_mul(shn[:], mean[:], rstd[:])
    shn_r = rows.tile([1, 512], F32R, tag="rr2")
    nc.vector.tensor_copy(shn_r[:], shn[:])
    o1 = p["ones128r"]
    p1 = p["pps"].tile([128, 512], F32, tag="bc")
    nc.tensor.matmul(p1[:], o1[0:1, :], rstd_r[:], start=True, stop=True)
    p2 = p["pps"].tile([128, 512], F32, tag="bc")
    nc.tensor.matmul(p2[:], o1[0:1, :], shn_r[:], start=True, stop=True)
    for c in range(16):
        eng = nc.gpsimd if c in GP_TILES else nc.vector
        eng.tensor_tensor(X[:, c, :], X[:, c, :], p1[:], OP.mult)
        eng.tensor_tensor(X[:, c, :], X[:, c, :], p2[:], OP.add)
        if finish is not None:
            finish(c, X)


def _proj_lhsT(nc, p, w_d, src, consume, wtag="w"):
    """psum[m] = sum_c w_d[m][:,c,:].T @ src[:,c,:]; consume(m, psum).
    src: [128,16,512] bf16 tile, or a list of 16 per-tile APs."""
    for m in range(16):
        wt = p["wp"].tile([128, 16, 128], BF16, tag=wtag)
        nc.sync.dma_start(wt[:], w_d[m])
        ps = p["pp"].tile([128, 512], F32, tag="p")
        for c in range(16):
            s = src[:, c, :] if not isinstance(src, list) else src[c][:]
            nc.tensor.matmul(ps[:], wt[:, c, :], s, start=(c == 0), stop=(c == 15))
        consume(m, ps)


def _attn_enc(nc, p, QT, KT, V, OT, maskb):
    """Distance-1 pipeline: scores/softmax prep of group g+1 issue before the
    AV matmuls of group g, so PE never waits on the reciprocal chain."""
    sel = p["sel"]
    stages = []   # per g: (Pg, [bc_psum x2])

    def scores(g):
        Pg = p["pgp"].tile([128, NH, 128], BF16, tag="Pg")
        for h in range(NH):
            S = p["pps"].tile([128, 512], F32, tag="S")
            for cc in (0, 1):
                nc.tensor.matmul(S[:, 0:128], KT[:, 2 * h + cc, ts(g, 128)],
                                 QT[:, 2 * h + cc, ts(g, 128)],
                                 start=(cc == 0), stop=(cc == 1))
            nc.scalar.activation(Pg[:, h, :], S[:, 0:128], AF.Exp, scale=SCALE)
        nc.vector.tensor_tensor(Pg[:], Pg[:], maskb[:], OP.mult)
        bcs = []
        for half in (0, 1):
            dn = p["ppr"].tile([1, 512], F32, tag="st")
            nc.tensor.matmul(dn[:], sel[:, 0:1], Pg[:, 4 * half:4 * half + 4, :],
                             start=True, stop=True)
            rc = p["rows"].tile([1, 512], F32, tag=f"r{half + 1}")
            nc.vector.reciprocal(rc[:], dn[:])
            rc_r = p["rows"].tile([1, 512], F32R, tag=f"rr{half + 1}")
            nc.vector.tensor_copy(rc_r[:], rc[:])
            bcp = p["pps"].tile([128, 512], F32, tag="bc")
            nc.tensor.matmul(bcp[:], p["ones128r"][0:1, :], rc_r[:],
                             start=True, stop=True)
            bcb = p["bcs"].tile([128, 512], BF16, tag="bcs")
            nc.vector.tensor_copy(bcb[:], bcp[:])
            bcs.append(bcb)
        stages.append((Pg, bcs))

    def av(g):
        Pg, bcs = stages[g]
        for h in range(NH):
            for mm in (0, 1):
                po = p["pps"].tile([128, 512], F32, tag="S")
                nc.tensor.matmul(po[:, 0:128], V[:, g, ds((2 * h + mm) * 128, 128)],
                                 Pg[:, h, :], start=True, stop=True)
                nc.vector.tensor_tensor(
                    OT[:, 2 * h + mm, ts(g, 128)], po[:, 0:128],
                    bcs[h // 4][:, ds((h % 4) * 128, 128)], OP.mult)

    scores(0)
    for g in range(1, 4):
        scores(g)
        av(g - 1)
    av(3)


def _attn_dec(nc, p, QT, KT, V, OT):
    sel = p["sel"]
    stages = []

    def scores(h):
        P = p["pgp"].tile([128, 4, 512], BF16, tag="Pd")
        for kt in range(4):
            S = p["pps"].tile([128, 512], F32, tag="S")
            for cc in (0, 1):
                nc.tensor.matmul(S[:], KT[:, 2 * h + cc, ts(kt, 128)],
                                 QT[:, 2 * h + cc, :], start=(cc == 0), stop=(cc == 1))
            nc.scalar.activation(P[:, kt, :], S[:], AF.Exp, scale=SCALE)
        dn = p["ppr"].tile([1, 512], F32, tag="st")
        for kt in range(4):
            nc.tensor.matmul(dn[:], sel[:, 0:1], P[:, kt, :],
                             start=(kt == 0), stop=(kt == 3))
        rc = p["rows"].tile([1, 512], F32, tag="r1")
        nc.vector.reciprocal(rc[:], dn[:])
        rc_r = p["rows"].tile([1, 512], F32R, tag="rr1")
        nc.vector.tensor_copy(rc_r[:], rc[:])
        bcp = p["pps"].tile([128, 512], F32, tag="bc")
        nc.tensor.matmul(bcp[:], p["ones128r"][0:1, :], rc_r[:], start=True, stop=True)
        bcb = p["bcs"].tile([128, 512], BF16, tag="bcs")
        nc.vector.tensor_copy(bcb[:], bcp[:])
        stages.append((P, bcb))

    def av(h):
        P, bcp = stages[h]
        for mm in (0, 1):
            po = p["pps"].tile([128, 512], F32, tag="S")
            for kt in range(4):
                nc.tensor.matmul(po[:], V[:, kt, ds((2 * h + mm) * 128, 128)],
                                 P[:, kt, :], start=(kt == 0), stop=(kt == 3))
            nc.vector.tensor_tensor(OT[:, 2 * h + mm, :], po[:], bcp[:], OP.mult)

    scores(0)
    for h in range(1, NH):
        scores(h)
        av(h - 1)
    av(NH - 1)


def build_phase(phase, n_layers=2, n_chunks=4, fusion=True, reps=1, split_waits=True):
    """phase: 'enc' or 'dec'. reps>1 wraps the whole body in a hardware loop
    (identical re-execution, for wall-clock timing of device time)."""
    enc = phase == "enc"
    Tn = CH * n_chunks
    nc = bass.Bass()
    x_d = nc.dram_tensor("x", [Dp, Tn], BF16, kind="ExternalInput")
    wd = {}
    for l in range(n_layers):
        for nm in ("wq", "wk", "wo", "w1", "w2"):
            shp = [16, 128, 16, 128]
            wd[f"{l}_{nm}"] = nc.dram_tensor(f"{phase}{l}_{nm}", shp, BF16,
                                             kind="ExternalInput")
        wd[f"{l}_wv"] = nc.dram_tensor(f"{phase}{l}_wv", [4, 128, 16, 512], BF16,
                                       kind="ExternalInput")
    if enc:
        mask_d = nc.dram_tensor("mask", [128, NH, 128], BF16, kind="ExternalInput")
        y_d = nc.dram_tensor("y", [Dp, Tn], BF16, kind="ExternalOutput")
    else:
        pos_d = nc.dram_tensor("pos", [Dp, B], BF16, kind="ExternalInput")
        if fusion:
            for nm in ("wfa", "wfb", "wa1", "wa2"):
                wd[nm] = nc.dram_tensor(nm, [16, 128, 16, 128], BF16,
                                        kind="ExternalInput")
            y_d = nc.dram_tensor("o", [2 * D, Tn], BF16, kind="ExternalOutput")
        else:
            y_d = nc.dram_tensor("y", [Dp, Tn], BF16, kind="ExternalOutput")

    from contextlib import ExitStack
    with tile.TileContext(nc) as tc, ExitStack() as ctx:
        p = {}
        const = ctx.enter_context(tc.tile_pool(name="const", bufs=1))
        p["xp"] = ctx.enter_context(tc.tile_pool(name="xp", bufs=1))
        p["xinp"] = ctx.enter_context(tc.tile_pool(name="xinp", bufs=1))
        p["castp"] = ctx.enter_context(tc.tile_pool(name="castp", bufs=1))
        p["bcs"] = ctx.enter_context(tc.tile_pool(name="bcs", bufs=4 if enc else 2))
        p["scrp"] = ctx.enter_context(tc.tile_pool(name="scrp", bufs=1))
        p["rbp"] = ctx.enter_context(tc.tile_pool(name="rbp", bufs=2))
        p["sqp"] = ctx.enter_context(tc.tile_pool(name="sqp", bufs=2))
        p["qtp"] = ctx.enter_context(tc.tile_pool(name="qtp", bufs=1))
        p["ktp"] = ctx.enter_context(tc.tile_pool(name="ktp", bufs=1))
        p["vp"] = ctx.enter_context(tc.tile_pool(name="vp", bufs=1))
        p["otp"] = ctx.enter_context(tc.tile_pool(name="otp", bufs=1))
        p["wp"] = ctx.enter_context(tc.tile_pool(name="wp", bufs=2))
        p["wvp"] = ctx.enter_context(tc.tile_pool(name="wvp", bufs=3))
        p["pgp"] = ctx.enter_context(tc.tile_pool(name="pgp", bufs=2))
        p["rows"] = ctx.enter_context(tc.tile_pool(name="rows", bufs=1))
        p["o2p"] = ctx.enter_context(tc.tile_pool(name="o2p", bufs=2))
        p["pp"] = ctx.enter_context(tc.tile_pool(name="pp", bufs=2, space="PSUM"))
        p["ppr"] = ctx.enter_context(tc.tile_pool(name="ppr", bufs=2, space="PSUM"))
        p["pps"] = ctx.enter_context(tc.tile_pool(name="pps", bufs=2, space="PSUM"))

        # constants
        sel = const.tile([128, 2], BF16)
        nc.vector.memset(sel[:, 0:1], 1.0)
        nc.vector.memset(sel[:, 1:2], 0.0)
        nc.vector.memset(sel[0:16, 1:2], 1.0)
        p["sel"] = sel
        onesf = const.tile([1, 128], F32)
        nc.vector.memset(onesf[:], 1.0)
        o128r = const.tile([1, 128], F32R)
        nc.vector.tensor_copy(o128r[:], onesf[:])
        p["ones128r"] = o128r
        epsr = const.tile([1, 1], F32)
        nc.vector.memset(epsr[:], EPS)
        p["epsr"] = epsr
        maskb = None
        if enc:
            maskb = const.tile([128, NH, 128], BF16)
            nc.sync.dma_start(maskb[:], mask_d[:])

        from contextlib import nullcontext
        loop_cm = tc.For_i(0, reps, 1) if reps > 1 else nullcontext()
        with loop_cm:
          for chk in range(n_chunks):
            # staged bf16 input (double-buffered: chunk k+1 loads during k)
            xin = p["xinp"].tile([128, 16, 512], BF16, tag="xin")
            nc.sync.dma_start(xin[:], _re(x_d[:, ts(chk, 512)]))
            X = p["xp"].tile([128, 16, 512], F32, tag="X")
            for c in range(16):
                nc.vector.tensor_copy(X[:, c, :], xin[:, c, :])  # f32 residual

            for l in range(n_layers):
                # ---- qkv inputs (V-projection runs first: for dec, the Q/K
                # input is built by adding pos in place into the y-cast after
                # V has consumed it, so one cast buffer serves the whole layer)
                if enc:
                    xqk = xv = xin if l == 0 else p["_cast_prev"]
                else:
                    xv = xin if l == 0 else p["_cast_prev"]

                V = p["vp"].tile([128, 4, Dp], BF16, tag="V")
                for n in range(4):
                    wta = p["wvp"].tile([128, 8, 512], BF16, tag="wv")
                    nc.sync.dma_start(wta[:], wd[f"{l}_wv"][n][:, ds(0, 8), :])
                    wtb = p["wvp"].tile([128, 8, 512], BF16, tag="wv")
                    nc.sync.dma_start(wtb[:], wd[f"{l}_wv"][n][:, ds(8, 8), :])
                    for mt in range(4):
                        ps = p["pp"].tile([128, 512], F32, tag="p")
                        for c in range(16):
                            wt = wta if c < 8 else wtb
                            nc.tensor.matmul(ps[:], xv[:, c, ts(mt, 128)],
                                             wt[:, c % 8, :],
                                             start=(c == 0), stop=(c == 15))
                        nc.vector.tensor_copy(V[:, mt, ts(n, 512)], ps[:])

                if not enc:
                    posb = p["scrp"].tile([128, 16, 512], BF16, tag="scr")
                    nc.sync.dma_start(posb[:], _re(pos_d[:]))
                    if l == 0:
                        xqk = p["castp"].tile([128, 16, 512], BF16, tag="cast")
                        for c in range(16):
                            nc.vector.tensor_tensor(xqk[:, c, :], xin[:, c, :],
                                                    posb[:, c, :], OP.add)
                    else:
                        xqk = xv   # in-place: xv tiles already consumed by V-proj
                        for c in range(16):
                            nc.vector.tensor_tensor(xqk[:, c, :], xqk[:, c, :],
                                                    posb[:, c, :], OP.add)

                QT = p["qtp"].tile([128, 16, 512], BF16, tag="QT")
                KT = p["ktp"].tile([128, 16, 512], BF16, tag="KT")
                _proj_lhsT(nc, p, wd[f"{l}_wq"], xqk,
                           lambda m, ps, _Q=QT: nc.vector.tensor_copy(_Q[:, m, :], ps[:]))
                _proj_lhsT(nc, p, wd[f"{l}_wk"], xqk,
                           lambda m, ps, _K=KT: nc.vector.tensor_copy(_K[:, m, :], ps[:]))

                OT = p["otp"].tile([128, 16, 512], BF16, tag="OT")
                if enc:
                    _attn_enc(nc, p, QT, KT, V, OT, maskb)
                else:
                    _attn_dec(nc, p, QT, KT, V, OT)

                # ---- out-proj + residual
                _proj_lhsT(nc, p, wd[f"{l}_wo"], OT,
                           lambda m, ps, _X=X: nc.vector.tensor_tensor(
                               _X[:, m, :], _X[:, m, :], ps[:], OP.add))
                # ---- LN1 (enc) / LN (dec): pipeline the FFN-input cast per tile
                tb = p["castp"].tile([128, 16, 512], BF16, tag="cast")
                def _fin_cast(c, _X, _tb=tb):
                    nc.scalar.activation(_tb[:, c, :], _X[:, c, :], AF.Copy)
                _ln_device(nc, p, X, finish=_fin_cast)
                # ---- FFN
                H = p["scrp"].tile([128, 16, 512], BF16, tag="scr")
                _proj_lhsT(nc, p, wd[f"{l}_w1"], tb,
                           lambda m, ps, _H=H: nc.scalar.activation(
                               _H[:, m, :], ps[:], AF.Relu))
                _proj_lhsT(nc, p, wd[f"{l}_w2"], H,
                           lambda m, ps, _X=X: nc.vector.tensor_tensor(
                               _X[:, m, :], _X[:, m, :], ps[:], OP.add))
                if enc:
                    last = l == n_layers - 1
                    if not last:
                        nxt = p["castp"].tile([128, 16, 512], BF16, tag="cast")
                        def _fin2(c, _X, _n=nxt):
                            nc.scalar.activation(_n[:, c, :], _X[:, c, :], AF.Copy)
                        _ln_device(nc, p, X, finish=_fin2)
                        p["_cast_prev"] = nxt
                    else:
                        # final LN: stream the bf16 output out per tile
                        def _fin_out(c, _X, _chk=chk):
                            ob = p["sqp"].tile([128, 512], BF16, tag="sq")
                            nc.scalar.activation(ob[:], _X[:, c, :], AF.Copy)
                            colsl = ts(_chk, 512)
                            if c < 15:
                                nc.sync.dma_start(
                                    y_d[ds(c * 128, 128), colsl], ob[:])
                            else:
                                nc.sync.dma_start(y_d[1920:1936, colsl], ob[0:16, :])
                        _ln_device(nc, p, X, finish=_fin_out)
                else:
                    if l < n_layers - 1:
                        nxt = p["castp"].tile([128, 16, 512], BF16, tag="cast")
                        for c in range(16):
                            nc.scalar.activation(nxt[:, c, :], X[:, c, :], AF.Copy)
                        p["_cast_prev"] = nxt

            if not enc and fusion:
                # ---------------- fusion head (chunk == one label, 512 occurrences)
                yb = p["castp"].tile([128, 16, 512], BF16, tag="cast")
                for c in range(16):
                    nc.scalar.activation(yb[:, c, :], X[:, c, :], AF.Copy)
                d0b = p["scrp"].tile([128, 16, 512], BF16, tag="scr")
                nc.vector.memset(d0b[:, :, 0:1], 0.0)
                nc.vector.tensor_copy(d0b[:, :, 1:512], yb[:, :, 0:511])

                diffb = p["qtp"].tile([128, 16, 512], BF16, tag="QT")
                for m in range(16):
                    wta = p["wp"].tile([128, 16, 128], BF16, tag="w")
                    nc.sync.dma_start(wta[:], wd["wfa"][m])
                    wtb = p["wp"].tile([128, 16, 128], BF16, tag="w")
                    nc.sync.dma_start(wtb[:], wd["wfb"][m])
                    ps = p["pp"].tile([128, 512], F32, tag="p")
                    for c in range(16):
                        nc.tensor.matmul(ps[:], wta[:, c, :], d0b[:, c, :],
                                         start=(c == 0), stop=False)
                    for c in range(16):
                        nc.tensor.matmul(ps[:], wtb[:, c, :], yb[:, c, :],
                                         start=False, stop=(c == 15))
                    nc.vector.tensor_copy(diffb[:, m, :], ps[:])

                t1b = p["ktp"].tile([128, 16, 512], BF16, tag="KT")
                _proj_lhsT(nc, p, wd["wa1"], diffb,
                           lambda m, ps, _t=t1b: nc.scalar.activation(
                               _t[:, m, :], ps[:], AF.Tanh))
                d2b = p["otp"].tile([128, 16, 512], BF16, tag="OT")
                _proj_lhsT(nc, p, wd["wa2"], t1b,
                           lambda m, ps, _t=d2b: nc.scalar.activation(
                               _t[:, m, :], ps[:], AF.Tanh))
                colsl = ts(chk, 512)
                nc.sync.dma_start(
                    y_d[0:1920, colsl].rearrange("(ci cp) t -> cp ci t", cp=128),
                    yb[:, 0:15, :])
                nc.sync.dma_start(y_d[1920:1936, colsl], yb[0:16, 15, :])
                for ci in range(16):
                    o2s = p["o2p"].tile([128, 512], BF16, tag="o2")
                    nc.vector.tensor_tensor(o2s[:, 1:512], d2b[:, ci, 1:512],
                                            X[:, ci, 0:511], OP.mult)
                    nc.vector.tensor_tensor(o2s[:, 0:1], d2b[:, ci, 0:1],
                                            X[:, ci, 0:1], OP.mult)
                    if ci < 15:
                        nc.sync.dma_start(
                            y_d[ds(1936 + ci * 128, 128), colsl], o2s[:])
                    else:
                        nc.sync.dma_start(y_d[3856:3872, colsl], o2s[0:16, :])

    if split_waits:
        _split_excess_waits(nc)
    return nc


# ----------------------------------------------------------------- host orchestration

_CACHE = {}

def _get_phase(phase, n_layers=2, n_chunks=4, fusion=True):
    key = (phase, n_layers, n_chunks, fusion)
    if key not in _CACHE:
        _CACHE[key] = build_phase(phase, n_layers, n_chunks, fusion)
    return _CACHE[key]


def _enc_inputs(w, feats):
    """feats: [B*K, D] f32. Returns per-core in_maps for phase 1."""
    FT = np.zeros((Dp, B * K), dtype=BF)
    FT[:D] = np.ascontiguousarray(feats.T).astype(BF)
    maps = []
    for c in range(NCORES):
        m = {"x": np.ascontiguousarray(FT[:, c * T:(c + 1) * T]), "mask": w["mask"]}
        for l in range(LENC):
            for nm in ("wq", "wk", "wv", "wo", "w1", "w2"):
                m[f"enc{l}_{nm}"] = w[f"enc{l}_{nm}"]
        maps.append(m)
    return maps


def _dec_inputs(w, enc_t):
    """enc_t: [Dp, B*K] bf16 (token-major i*K+j). Returns per-core in_maps."""
    E = enc_t.reshape(Dp, B, K)
    maps = []
    for c in range(NCORES):
        Y = np.ascontiguousarray(
            E[:, :, c * 4:(c + 1) * 4].transpose(0, 2, 1)).reshape(Dp, T)
        m = {"x": Y, "pos": w["pos"]}
        for l in range(LDEC):
            for nm in ("wq", "wk", "wv", "wo", "w1", "w2"):
                m[f"dec{l}_{nm}"] = w[f"dec{l}_{nm}"]
        for nm in ("wfa", "wfb", "wa1", "wa2"):
            m[nm] = w[nm]
        maps.append(m)
    return maps


def kernel(**inputs):
    inp = {k: np.asarray(v) for k, v in inputs.items()}
    feats = inp["features"].astype(np.float32)
    w = _prep_weights(inp)

    nc1 = _get_phase("enc")
    maps1 = _enc_inputs(w, feats)
    res1 = run_bass_kernel_spmd(nc1, maps1, core_ids=list(range(NCORES)))
    enc_t = np.concatenate([res1.results[c]["y"] for c in range(NCORES)], axis=1)

    nc2 = _get_phase("dec")
    maps2 = _dec_inputs(w, enc_t)
    res2 = run_bass_kernel_spmd(nc2, maps2, core_ids=list(range(NCORES)))

    out = np.empty((B * K, 2 * D), dtype=np.float32)
    out_v = out.reshape(B, K, 2 * D)
    for c in range(NCORES):
        O = res2.results[c]["o"].astype(np.float32).reshape(2 * D, 4, B)
        out_v[:, c * 4:(c + 1) * 4, :] = O.transpose(2, 1, 0)
    return out
